# revision 56
# baseline (speedup 1.0000x reference)
"""Trainium2 Bass kernel for nn_DisCA (dual conv-block + channel attention).

Data-parallel over batch: 8 batch items -> 8 NeuronCores, one image per core.
Conv weights / BN affine replicated. BatchNorm batch statistics are obtained
with a per-block cross-core AllReduce of per-channel (sum, sumsq) ([1,1024]
f32 each). The BN affine transform is folded algebraically into the
attention-score matrix,
    scores = a1[c]*a2[d]*S[c,d] + (a1*r1)[c]*b2bn[d] + b1bn[c]*(a2*r2+N*b2bn)[d]
where S is the raw (pre-BN) Gram matrix and r_i are local per-channel row
sums, so the raw score matmul overlaps the AllReduce latency.

v2 layout/schedule notes:
  - y1c holds conv1's output once, zero-padded to 34x34 per channel chunk;
    conv2's stationary operands are strided [4,32] windows into it (no
    shifted copies).
  - conv2 runs s-outer so LeakyReLU / Square / stats matmuls interleave with
    the conv matmuls and the stats AllReduce triggers right after the last
    conv2 matmul.
  - block-1 BN affine params are computed (free layout) right after AR1,
    hidden under block-2's conv; block-2 params go through a per-partition
    [128,4] pipeline built from tiny PE transpose matmuls so the post-AR2
    serial chain is short.
  - all tail matmuls use the f32r single-pass PE mode.
"""

import os
import sys

for _p in ("/opt/trn_rl_repo", "/root/.axon_site/_ro/trn_rl_repo"):
    if os.path.isdir(_p) and _p not in sys.path:
        sys.path.insert(0, _p)

import numpy as np
import ml_dtypes

import concourse.bacc as bacc
import concourse.mybir as mybir
from concourse.tile import TileContext, add_dep_helper
from concourse.bass_utils import run_bass_kernel_spmd
from concourse.masks import make_identity

F32 = mybir.dt.float32
F32R = mybir.dt.float32r
BF16 = mybir.dt.bfloat16


def _r(ap):
    """Reinterpret an fp32 AP as float32r (single-pass full-rate PE mode)."""
    return ap.bitcast(F32R)


AF = mybir.ActivationFunctionType
ALU = mybir.AluOpType

NCORES = 8
B, C, H, W = 8, 512, 32, 32
N = H * W                      # 1024 spatial positions per image
CMID = 256                     # conv1 output channels
HP = H + 2                     # padded spatial dim (34)
NPAD = HP * HP                 # 1156 elems per channel chunk in y1c
BN_EPS = 1e-5
LRELU_SLOPE = 0.01
M_TOTAL = float(B * N)         # BN statistic count (full batch)

KC = C // 128                  # 4 channel chunks of x
KM = CMID // 128               # 2 channel chunks of mid features


def build_kernel():
    nc = bacc.Bacc("TRN2", target_bir_lowering=False, debug=False,
                   num_devices=NCORES)

    # ---- DRAM I/O -------------------------------------------------------
    x1d = nc.dram_tensor("x1s", [128, 4096], F32, kind="ExternalInput")
    x2d = nc.dram_tensor("x2s", [128, 4096], F32, kind="ExternalInput")
    xd = nc.dram_tensor("xs", [128, 4096], F32, kind="ExternalInput")
    w1d = nc.dram_tensor("w1t", [128, 1024], F32, kind="ExternalInput")
    w2d = nc.dram_tensor("w2t", [128, 9216], F32, kind="ExternalInput")
    # vecs rows: 0=b2, 1=gamma, 2=bn_bias, 3=beta(col0), 4=b1(cols 0:256)
    vecd = nc.dram_tensor("vecs", [8, 512], F32, kind="ExternalInput")
    outd = nc.dram_tensor("out", [128, 4096], F32, kind="ExternalOutput")

    cc_in = nc.dram_tensor("cc_in", [1, 2048], F32, kind="Internal")
    cc_out = nc.dram_tensor("cc_out", [1, 2048], F32, kind="Internal",
                            addr_space="Shared")
    cw_in = nc.dram_tensor("cw_in", [1, 8], F32, kind="Internal")
    cw_out = nc.dram_tensor("cw_out", [1, 8], F32, kind="Internal",
                            addr_space="Shared")

    with TileContext(nc, num_cores=NCORES) as tc:
        with (
            tc.tile_pool(name="const", bufs=1) as const,
            tc.tile_pool(name="big", bufs=1) as big,
            tc.tile_pool(name="work", bufs=2) as work,
            tc.tile_pool(name="vec", bufs=1) as vec,
            tc.tile_pool(name="ps", bufs=6, space="PSUM") as ps,
            tc.tile_pool(name="psS", bufs=2, space="PSUM") as psS,
        ):
            # ---- input DMAs first: conv1's operands lead ---------------
            w1t = big.tile([128, 1024], F32)
            nc.sync.dma_start(out=_r(w1t[:]), in_=_r(w1d[:]))
            x1s = work.tile([128, 4096], F32, tag="xin")
            nc.sync.dma_start(out=_r(x1s[:, 0:1024]), in_=_r(x1d[:, 0:1024]))
            nc.sync.dma_start(out=_r(x1s[:, 1024:4096]),
                              in_=_r(x1d[:, 1024:4096]))

            # ---- constants / small tiles -------------------------------
            identity = const.tile([128, 128], F32)
            make_identity(nc, identity)
            ones_col = const.tile([128, 1], F32)   # lhsT for partition sums
            nc.vector.memset(ones_col[:], 1.0)
            nc.scalar.copy(_r(ones_col[:]), ones_col[:])
            ones_row = const.tile([1, 128], F32)   # K=1 lhsT (f32 mms)
            nc.vector.memset(ones_row[:], 1.0)
            ones_row_r = const.tile([1, 128], F32)  # K=1 lhsT (f32r mms)
            nc.vector.memset(ones_row_r[:], 1.0)
            nc.scalar.copy(_r(ones_row_r[:]), ones_row_r[:])
            one_one = const.tile([1, 1], F32)      # rhs for transpose-in mms
            nc.vector.memset(one_one[:], 1.0)
            eps_pp = const.tile([128, 1], F32)     # BN eps as bias APs
            nc.vector.memset(eps_pp[:], BN_EPS)
            eps_1 = const.tile([1, 1], F32)
            nc.vector.memset(eps_1[:], BN_EPS)
            zrow = const.tile([128, 32], F32)      # zero source for borders
            nc.vector.memset(zrow[:], 0.0)
            zrowf = const.tile([1, 8], F32)
            nc.vector.memset(zrowf[:], 0.0)
            # preload every ACT table now (idle head) so no 1.3us table
            # switch lands mid-stream on the critical path later
            tld = const.tile([1, 8], F32)
            nc.scalar.activation(tld[:], zrowf[:], AF.Exp)
            nc.scalar.activation(tld[:], zrowf[:], AF.Sqrt)
            nc.scalar.activation(tld[:], zrowf[:], AF.Square)
            nc.scalar.activation(tld[:], zrowf[:], AF.Lrelu,
                                 alpha=LRELU_SLOPE)

            # conv-critical inputs next: b1 (conv1 writes), b2 (conv2
            # bias), then the big w2t whose first chunk gates conv2 s=0
            b1pp = const.tile([128, KM], F32)      # b1 per-partition chunks
            for m in range(KM):
                nc.sync.dma_start(out=b1pp[:, m:m + 1],
                                  in_=vecd[4:5, 128 * m:128 * (m + 1)])
            b2row = const.tile([1, 512], F32)      # matmul rhs (f32r)
            nc.sync.dma_start(out=_r(b2row[:]), in_=_r(vecd[0:1, :]))
            w2t = big.tile([128, 9216], F32)
            for j in range(3):
                nc.sync.dma_start(out=_r(w2t[:, 3072 * j:3072 * (j + 1)]),
                                  in_=_r(w2d[:, 3072 * j:3072 * (j + 1)]))

            # tiny warmup AllReduce: pays the collective setup cost while
            # the input DMAs stream, so the real stats ARs hit a warm path
            warm = const.tile([1, 8], F32)
            nc.vector.memset(warm[:], 1.0)
            nc.sync.dma_start(out=cw_in[:], in_=warm[:])
            nc.gpsimd.collective_compute(
                "AllReduce", ALU.add,
                replica_groups=[list(range(NCORES))],
                ins=[cw_in[:]], outs=[cw_out[:]])

            x2s = work.tile([128, 4096], F32, tag="xin")
            for j in range(2):
                nc.sync.dma_start(out=_r(x2s[:, 2048 * j:2048 * (j + 1)]),
                                  in_=_r(x2d[:, 2048 * j:2048 * (j + 1)]))
            # affine-math vectors are only needed post-AR1 (~100us in)
            gb = const.tile([1, 1024], F32)        # gamma | bn_bias (f32)
            nc.sync.dma_start(out=gb[0:1, 0:512], in_=vecd[1:2, :])
            nc.sync.dma_start(out=gb[0:1, 512:1024], in_=vecd[2:3, :])
            betar = const.tile([1, 1], F32)
            nc.sync.dma_start(out=betar[:], in_=vecd[3:4, 0:1])
            # gamma / bn_bias per-partition [128, 4] (partition-scatter)
            gpp = const.tile([128, KC], F32)
            nc.sync.dma_start(
                out=gpp[:],
                in_=vecd[1:2, :].rearrange("o (j p) -> o p j", p=128))
            bnbpp = const.tile([128, KC], F32)
            nc.sync.dma_start(
                out=bnbpp[:],
                in_=vecd[2:3, :].rearrange("o (j p) -> o p j", p=128))

            # conv1 output, 3 horizontally-pre-shifted copies (kw = 0,1,2),
            # each vertically zero-padded to 34 rows of 32 contiguous cols:
            #   y1c[kw][k][c, r, w] = Y1[c_chunk k][r-1, w + kw - 1]
            # so conv2's stationary operand (s, kh, kw) is the contiguous
            # 128-elem slice at rows 4s+kh .. 4s+kh+3 of copy kw.
            NROW = HP * W                       # 1088 elems per copy/chunk
            y1c = big.tile([128, 3 * KM * NROW], F32)

            def y1base(kw, k):
                return (kw * KM + k) * NROW

            def ycv(kw, k):
                return y1c[:, y1base(kw, k):y1base(kw, k) + NROW].rearrange(
                    "p (r c) -> p r c", c=W)
            # zero only the borders (interior is fully overwritten by conv1)
            zr = zrow[:].rearrange("p (a c) -> p a c", a=1)      # [128,1,32]
            zc = zrow[:].rearrange("p (c a) -> p c a", a=1)      # [128,32,1]
            for kw in range(3):
                for k in range(KM):
                    v = ycv(kw, k)
                    nc.scalar.copy(_r(v[:, 0:1, :]), zr)         # top row
                    nc.scalar.copy(_r(v[:, HP - 1:HP, :]), zr)   # bottom
                    if kw == 0:
                        nc.scalar.copy(_r(v[:, 1:HP - 1, 0:1]), zc)
                    if kw == 2:
                        nc.scalar.copy(_r(v[:, 1:HP - 1, W - 1:W]), zc)

            f1t = big.tile([128, 4096], F32)
            f2t = big.tile([128, 4096], F32)
            stats = const.tile([1, 2048], F32)     # local r1|s1|r2|s2
            r2lrow = const.tile([1, 512], F32)     # local r2, f32r for PE
            ar = const.tile([1, 2048], F32)        # all-reduced stats

            # ---- one conv block: x -> conv1 -> pad -> conv2 -> lrelu ---
            def conv_block(xin, ft, si):
                # conv1: Y1[cmid, n] = W1 @ x + b1, written into shifted
                # copies of y1c (n2-outer: conv2's early s-chunks only need
                # the n2=0 rows, so they start after 6 writes instead of 12)
                for n2 in range(2):
                    for m in range(KM):
                        acc = ps.tile([128, 512], F32, tag="ps")
                        for k in range(KC):
                            nc.tensor.matmul(
                                acc[:],
                                _r(w1t[:, 256 * k + 128 * m:256 * k + 128 * (m + 1)]),
                                _r(xin[:, 1024 * k + 512 * n2:1024 * k + 512 * (n2 + 1)]),
                                start=(k == 0), stop=(k == KC - 1))
                        accv = acc[:].rearrange("p (r c) -> p r c", c=W)
                        row0 = (1 + 16 * n2) * W
                        # center copy (kw=1): straight contiguous store
                        nc.scalar.activation(
                            _r(y1c[:, y1base(1, m) + row0:y1base(1, m) + row0 + 512]),
                            acc[:], AF.Identity, bias=b1pp[:, m:m + 1])
                        # kw=0: shift right one col (src cols 0..30 -> 1..31)
                        d0 = ycv(0, m)
                        nc.scalar.activation(
                            _r(d0[:, 1 + 16 * n2:17 + 16 * n2, 1:32]),
                            accv[:, :, 0:31], AF.Identity,
                            bias=b1pp[:, m:m + 1])
                        # kw=2: shift left one col (src cols 1..31 -> 0..30)
                        d2 = ycv(2, m)
                        nc.scalar.activation(
                            _r(d2[:, 1 + 16 * n2:17 + 16 * n2, 0:31]),
                            accv[:, :, 1:32], AF.Identity,
                            bias=b1pp[:, m:m + 1])

                # conv2 (3x3) -> transposed output F^T[n, c], s-outer so the
                # activations + stats interleave with the conv matmuls
                racc = psS.tile([1, 512], F32, tag="stat")
                qacc = psS.tile([1, 512], F32, tag="stat")
                for s in range(8):
                    acc = ps.tile([128, 512], F32, tag="ps")
                    # bias: + b2[c] on every row (rank-1, K=1)
                    nc.tensor.matmul(acc[:], _r(ones_row_r[:]), _r(b2row[:]),
                                     start=True, stop=False)
                    for kh in range(3):
                        for kw in range(3):
                            t = kh * 3 + kw
                            for k in range(KM):
                                off = y1base(kw, k) + (4 * s + kh) * W
                                rhs = w2t[:, (2 * t + k) * 512:(2 * t + k + 1) * 512]
                                last = (kh == 2 and kw == 2 and k == KM - 1)
                                nc.tensor.matmul(acc[:],
                                                 _r(y1c[:, off:off + 128]),
                                                 _r(rhs),
                                                 start=False, stop=last)
                    nc.scalar.activation(_r(ft[:, 512 * s:512 * (s + 1)]),
                                         acc[:], AF.Lrelu,
                                         alpha=LRELU_SLOPE)
                    sq = work.tile([128, 512], F32, tag="sq")
                    nc.scalar.activation(_r(sq[:]),
                                         ft[:, 512 * s:512 * (s + 1)],
                                         AF.Square)
                    nc.tensor.matmul(racc[:], _r(ones_col[:]),
                                     _r(ft[:, 512 * s:512 * (s + 1)]),
                                     start=(s == 0), stop=(s == 7))
                    nc.tensor.matmul(qacc[:], _r(ones_col[:]), _r(sq[:]),
                                     start=(s == 0), stop=(s == 7))

                # local stats -> sbuf + AllReduce for this block
                nc.scalar.copy(stats[0:1, 1024 * si:1024 * si + 512], racc[:])
                nc.scalar.copy(stats[0:1, 1024 * si + 512:1024 * (si + 1)],
                               qacc[:])
                if si == 1:
                    nc.scalar.copy(r2lrow[:], racc[:])
                nc.sync.dma_start(out=cc_in[0:1, 1024 * si:1024 * (si + 1)],
                                  in_=stats[0:1, 1024 * si:1024 * (si + 1)])
                nc.gpsimd.collective_compute(
                    "AllReduce", ALU.add,
                    replica_groups=[list(range(NCORES))],
                    ins=[cc_in[0:1, 1024 * si:1024 * (si + 1)]],
                    outs=[cc_out[0:1, 1024 * si:1024 * (si + 1)]])
                if si == 1:
                    nc.sync.dma_start(out=ar[0:1, 1024:2048],
                                      in_=cc_out[0:1, 1024:2048])

            def tail(xs, r1pp, s1pp):
                # ---- raw Gram matmuls (no AR dependency) ---------------
                ssb = big.tile([128, 2048], F32)
                for m in range(KC):
                    sacc = ps.tile([128, 512], F32, tag="ps")
                    for k in range(8):
                        nc.tensor.matmul(
                            sacc[:],
                            _r(f1t[:, 512 * k + 128 * m:512 * k + 128 * (m + 1)]),
                            _r(f2t[:, 512 * k:512 * (k + 1)]),
                            start=(k == 0), stop=(k == 7))
                    nc.vector.tensor_copy(ssb[:, 512 * m:512 * (m + 1)],
                                          sacc[:])

                # ---- block-1 affine params, per-partition layout (AR1
                # landed long ago; runs on DVE concurrently with the Gram;
                # the scatter DMAs were issued between the conv blocks so
                # they are not queued behind the AR2-blocked ar dma) -------
                a1pp = vec.tile([128, KC], F32)
                nc.vector.tensor_scalar_mul(r1pp[:], r1pp[:], 1.0 / M_TOTAL)
                nc.vector.tensor_mul(a1pp[:], r1pp[:], r1pp[:])    # mean^2
                nc.vector.tensor_scalar(s1pp[:], s1pp[:], 1.0 / M_TOTAL,
                                        BN_EPS, op0=ALU.mult, op1=ALU.add)
                nc.vector.tensor_sub(s1pp[:], s1pp[:], a1pp[:])    # var+eps
                nc.scalar.activation(s1pp[:], s1pp[:], AF.Sqrt)
                nc.vector.reciprocal(s1pp[:], s1pp[:])
                nc.vector.tensor_mul(a1pp[:], gpp[:], s1pp[:])
                # b1bn per-partition = bn_bias - mean1*a1 (r1pp holds mean1)
                b1bnpp = vec.tile([128, KC], F32)
                nc.vector.tensor_mul(b1bnpp[:], r1pp[:], a1pp[:])
                nc.vector.tensor_sub(b1bnpp[:], bnbpp[:], b1bnpp[:])

                # local r1, r2 into per-partition layout (pre-AR2, idle PE)
                r2lps = ps.tile([128, 2 * KC], F32, tag="ps")
                for j in range(KC):
                    nc.tensor.matmul(r2lps[:, j:j + 1],
                                     stats[0:1, 128 * j:128 * (j + 1)],
                                     one_one[:], start=True, stop=True)
                    nc.tensor.matmul(r2lps[:, KC + j:KC + j + 1],
                                     r2lrow[0:1, 128 * j:128 * (j + 1)],
                                     one_one[:], start=True, stop=True)
                rlpp = vec.tile([128, 2 * KC], F32)
                nc.vector.tensor_copy(rlpp[:], r2lps[:])
                r2lpp = rlpp[:, KC:2 * KC]
                # u per-partition = a1 * r1_local
                u_pp = vec.tile([128, KC], F32)
                nc.vector.tensor_mul(u_pp[:], a1pp[:], rlpp[:, 0:KC])

                tld2 = vec.tile([1, 8], F32)
                # beta broadcast [128,1]
                bps = ps.tile([128, 1], F32, tag="ps")
                nc.tensor.matmul(bps[:], ones_row[:], betar[:],
                                 start=True, stop=True)
                betapp = vec.tile([128, 1], F32)
                nc.vector.tensor_copy(betapp[:], bps[:])

                # HAM keep-warm fillers bridging the AR2 latency window
                # (~0.26us each; the recent-measured collective latency is
                # 20-30us and the Gram covers only ~8.5us of it)
                for _ in range(48):
                    wf = psS.tile([128, 512], F32, tag="stat")
                    nc.tensor.matmul(wf[:], _r(ones_row_r[:]), _r(b2row[:]),
                                     start=True, stop=True)

                # scheduler fence: nothing below may be reordered above this
                # point, so the AR2-dependent matmuls cannot hoist ahead and
                # stall the in-order PE queue
                tc.no_sync_barrier()

                # ---- block-2 affine params, per-partition pipeline ------
                # transpose-in: ar2 (r2|s2 global) -> [128, 8] psum
                ppb = ps.tile([128, 8], F32, tag="ps")
                for j in range(8):
                    nc.tensor.matmul(ppb[:, j:j + 1],
                                     ar[0:1, 1024 + 128 * j:1024 + 128 * (j + 1)],
                                     one_one[:], start=True, stop=True)
                pps = vec.tile([128, 8], F32)
                nc.vector.tensor_copy(pps[:], ppb[:])
                r2pp = pps[:, 0:4]
                s2pp = pps[:, 4:8]
                # ppout cols: 0:4 a2 | 4:8 b2bn | 8:12 w
                ppout = vec.tile([128, 12], F32)
                ppt = vec.tile([128, 8], F32)
                mean2 = ppt[:, 0:4]
                var2 = ppt[:, 4:8]
                nc.vector.tensor_scalar_mul(mean2, r2pp, 1.0 / M_TOTAL)
                nc.vector.tensor_mul(var2, mean2, mean2)
                nc.vector.scalar_tensor_tensor(
                    var2, s2pp, 1.0 / M_TOTAL, var2,
                    op0=ALU.mult, op1=ALU.subtract)
                sd2 = ppout[:, 8:12]
                nc.scalar.activation(sd2, var2, AF.Sqrt, bias=eps_pp[:])
                nc.scalar.activation(tld2[:], zrowf[:], AF.Exp)
                nc.vector.reciprocal(var2, sd2)                    # rstd
                a2pp = ppout[:, 0:4]
                b2pp = ppout[:, 4:8]
                wpp = ppout[:, 8:12]
                nc.vector.tensor_mul(a2pp, var2, gpp[:])           # a2
                nc.vector.tensor_mul(var2, mean2, a2pp)            # m*a2
                nc.vector.tensor_sub(b2pp, bnbpp[:], var2)         # b2bn
                nc.vector.tensor_mul(var2, a2pp, r2lpp[:])         # a2*r2loc
                nc.vector.scalar_tensor_tensor(
                    wpp, b2pp, float(N), var2,
                    op0=ALU.mult, op1=ALU.add)                     # w
                # a2, b2bn, w -> free-layout [1,512] rows (4 matmuls each):
                # out[0, 128j+q] = sum_p ppout[p, col] * I[p, q]
                rows3 = vec.tile([1, 1536], F32)   # a2 | b2bn | w
                for v in range(3):
                    vps = ps.tile([1, 512], F32, tag="ps")
                    for j in range(KC):
                        nc.tensor.matmul(vps[0:1, 128 * j:128 * (j + 1)],
                                         ppout[:, 4 * v + j:4 * v + j + 1],
                                         identity[:],
                                         start=True, stop=True)
                    nc.scalar.copy(rows3[0:1, 512 * v:512 * (v + 1)], vps[:])

                # broadcasts of a2 / b2bn / w to [128,512] (exact f32 mms;
                # copied to SBUF so the PSUM banks free up for the m-loop)
                bc3 = vec.tile([128, 1536], F32)   # a2b | b2bc | wbc
                for v in range(3):
                    bcp = ps.tile([128, 512], F32, tag="ps")
                    nc.tensor.matmul(bcp[:], ones_row[:],
                                     rows3[0:1, 512 * v:512 * (v + 1)],
                                     start=True, stop=True)
                    nc.vector.tensor_copy(bc3[:, 512 * v:512 * (v + 1)],
                                          bcp[:])
                a2b = bc3[:, 0:512]
                b2bc = bc3[:, 512:1024]
                wbc = bc3[:, 1024:1536]
                # keep the PE warm while the m=0 DVE chain fills
                for _ in range(12):
                    wf = psS.tile([128, 512], F32, tag="stat")
                    nc.tensor.matmul(wf[:], _r(ones_row_r[:]), _r(b2row[:]),
                                     start=True, stop=True)

                # ---- scores + softmax-exp + transpose + apply ----------
                et = y1c                 # E^T overlaid on dead y1c
                scvec = vec.tile([128, KC], F32)
                for m in range(KC):
                    # rank-1 score corrections (exact f32):
                    #   rk = u[c]*b2bn[d] + b1bn[c]*w[d]
                    rk = work.tile([128, 512], F32, tag="rk")
                    nc.vector.tensor_scalar(rk[:], b2bc, u_pp[:, m:m + 1],
                                            None, op0=ALU.mult)
                    nc.vector.scalar_tensor_tensor(
                        rk[:], wbc, b1bnpp[:, m:m + 1], rk[:],
                        op0=ALU.mult, op1=ALU.add)
                    # scores = (S * a2[d]) * a1[c] + rk  (the plain mul runs
                    # on GpSimd to keep the DVE free for the softmax chain)
                    tmul = work.tile([128, 512], F32, tag="tmul")
                    nc.gpsimd.tensor_mul(tmul[:], ssb[:, 512 * m:512 * (m + 1)],
                                         a2b)
                    sc = work.tile([128, 512], F32, tag="scores")
                    nc.vector.scalar_tensor_tensor(
                        sc[:], tmul[:], a1pp[:, m:m + 1], rk[:],
                        op0=ALU.mult, op1=ALU.add)
                    # E = exp(scores - rowmax), sumexp accumulated for free
                    nmx = vec.tile([128, 1], F32, tag="nmx")
                    nc.vector.tensor_reduce(nmx[:], sc[:],
                                            axis=mybir.AxisListType.X,
                                            op=ALU.max, negate=True)
                    esum = vec.tile([128, 1], F32, tag="esum")
                    ee = work.tile([128, 512], F32, tag="ee")
                    nc.scalar.activation(ee[:], sc[:], AF.Exp, bias=nmx[:],
                                         accum_out=esum[:])
                    nc.vector.reciprocal(esum[:], esum[:])
                    nc.vector.tensor_mul(scvec[:, m:m + 1], esum[:], betapp[:])
                    # transpose E chunk into et
                    for j in range(KC):
                        tp = ps.tile([128, 128], F32, tag="ps")
                        nc.tensor.transpose(tp[:], ee[:, 128 * j:128 * (j + 1)],
                                            identity[:])
                        nc.scalar.copy(
                            _r(et[:, 512 * j + 128 * m:512 * j + 128 * (m + 1)]),
                            tp[:])
                    # out[c, n] = (beta/sumexp)[c] * sum_d E^T[d,c] x[d,n]
                    for n2 in range(2):
                        oacc = ps.tile([128, 512], F32, tag="ps")
                        for k in range(KC):
                            nc.tensor.matmul(
                                oacc[:],
                                _r(et[:, 512 * k + 128 * m:512 * k + 128 * (m + 1)]),
                                _r(xs[:, 1024 * k + 512 * n2:1024 * k + 512 * (n2 + 1)]),
                                start=(k == 0), stop=(k == KC - 1))
                        ot = work.tile([128, 512], F32, tag="ot")
                        nc.scalar.mul(ot[:], oacc[:], scvec[:, m:m + 1])
                        nc.sync.dma_start(
                            out=outd[:, 1024 * m + 512 * n2:1024 * m + 512 * (n2 + 1)],
                            in_=ot[:])
                    if m < KC - 1:
                        for _ in range(8):
                            wf = psS.tile([128, 512], F32, tag="stat")
                            nc.tensor.matmul(wf[:], _r(ones_row_r[:]),
                                             _r(b2row[:]),
                                             start=True, stop=True)

            conv_block(x1s, f1t, 0)
            # x reuses x1's slot (x1 is dead after its conv1)
            xs = work.tile([128, 4096], F32, tag="xin")
            for j in range(2):
                nc.sync.dma_start(out=_r(xs[:, 2048 * j:2048 * (j + 1)]),
                                  in_=_r(xd[:, 2048 * j:2048 * (j + 1)]))
            # block-1 global-stats scatter loads, issued HERE so they sit
            # ahead of AR2's blocking ar-dma in the in-order sync queue
            # (they only wait on AR1, which completes during conv_block(x2))
            r1pp = vec.tile([128, KC], F32)
            s1pp = vec.tile([128, KC], F32)
            nc.sync.dma_start(
                out=r1pp[:],
                in_=cc_out[0:1, 0:512].rearrange("o (j p) -> o p j", p=128))
            nc.sync.dma_start(
                out=s1pp[:],
                in_=cc_out[0:1, 512:1024].rearrange("o (j p) -> o p j", p=128))
            conv_block(x2s, f2t, 1)
            tail(xs, r1pp, s1pp)

    nc.compile()
    return nc


_NC_CACHE = []


def _get_nc():
    if not _NC_CACHE:
        _NC_CACHE.append(build_kernel())
    return _NC_CACHE[0]


def _prep_shared(w1, b1, w2, b2, gamma, bn_bias, beta):
    w1m = w1.reshape(CMID, C).astype(np.float32)
    w1t = np.ascontiguousarray(
        w1m.T.reshape(KC, 128, CMID).transpose(1, 0, 2).reshape(128, KC * CMID))
    w2t = np.empty((128, 9216), dtype=np.float32)
    for kh in range(3):
        for kw in range(3):
            t = kh * 3 + kw
            wt = w2[:, :, kh, kw].T  # [256 in, 512 out]
            for k in range(KM):
                w2t[:, (2 * t + k) * 512:(2 * t + k + 1) * 512] = \
                    wt[128 * k:128 * (k + 1), :]
    vecs = np.zeros((8, 512), dtype=np.float32)
    vecs[0] = b2
    vecs[1] = gamma
    vecs[2] = bn_bias
    vecs[3, 0] = np.asarray(beta).reshape(-1)[0]
    vecs[4, :CMID] = b1
    return w1t, w2t, vecs


def _chunk_img(img):
    # [512, 1024] -> [128, 4096] with channel chunk k at cols 1024k
    return np.ascontiguousarray(
        img.reshape(KC, 128, N).transpose(1, 0, 2).reshape(128, KC * N))


def kernel(x, x1, x2, w1, b1, w2, b2, gamma, bn_bias, beta, **run_kw):
    nc = _get_nc()
    w1t, w2t, vecs = _prep_shared(w1, b1, w2, b2, gamma, bn_bias, beta)
    in_maps = []
    for i in range(NCORES):
        in_maps.append({
            "x1s": _chunk_img(np.asarray(x1[i], np.float32).reshape(C, N)),
            "x2s": _chunk_img(np.asarray(x2[i], np.float32).reshape(C, N)),
            "xs": _chunk_img(np.asarray(x[i], np.float32).reshape(C, N)),
            "w1t": w1t, "w2t": w2t, "vecs": vecs,
        })
    res = run_bass_kernel_spmd(nc, in_maps, list(range(NCORES)), **run_kw)
    out = np.empty((B, C, H, W), dtype=np.float32)
    for i in range(NCORES):
        o = res.results[i]["out"]  # [128, 4096]
        out[i] = o.reshape(128, KC, N).transpose(1, 0, 2).reshape(C, H, W)
    if run_kw:
        kernel.last_results = res
    return out


# revision 58
# speedup vs baseline: 1.0580x; 1.0580x over previous
"""Trainium2 Bass kernel for nn_DisCA (dual conv-block + channel attention).

Data-parallel over batch: 8 batch items -> 8 NeuronCores, one image per core.
Conv weights / BN affine replicated. BatchNorm batch statistics are obtained
with a per-block cross-core AllReduce of per-channel (sum, sumsq) ([1,1024]
f32 each). The BN affine transform is folded algebraically into the
attention-score matrix,
    scores = a1[c]*a2[d]*S[c,d] + (a1*r1)[c]*b2bn[d] + b1bn[c]*(a2*r2+N*b2bn)[d]
where S is the raw (pre-BN) Gram matrix and r_i are local per-channel row
sums, so the raw score matmul overlaps the AllReduce latency.

v2 layout/schedule notes:
  - y1c holds conv1's output once, zero-padded to 34x34 per channel chunk;
    conv2's stationary operands are strided [4,32] windows into it (no
    shifted copies).
  - conv2 runs s-outer so LeakyReLU / Square / stats matmuls interleave with
    the conv matmuls and the stats AllReduce triggers right after the last
    conv2 matmul.
  - block-1 BN affine params are computed (free layout) right after AR1,
    hidden under block-2's conv; block-2 params go through a per-partition
    [128,4] pipeline built from tiny PE transpose matmuls so the post-AR2
    serial chain is short.
  - all tail matmuls use the f32r single-pass PE mode.
"""

import os
import sys

for _p in ("/opt/trn_rl_repo", "/root/.axon_site/_ro/trn_rl_repo"):
    if os.path.isdir(_p) and _p not in sys.path:
        sys.path.insert(0, _p)

import numpy as np
import ml_dtypes

import concourse.bacc as bacc
import concourse.mybir as mybir
from concourse.tile import TileContext, add_dep_helper
from concourse.bass_utils import run_bass_kernel_spmd
from concourse.masks import make_identity

F32 = mybir.dt.float32
F32R = mybir.dt.float32r
BF16 = mybir.dt.bfloat16


def _r(ap):
    """Reinterpret an fp32 AP as float32r (single-pass full-rate PE mode)."""
    return ap.bitcast(F32R)


AF = mybir.ActivationFunctionType
ALU = mybir.AluOpType

NCORES = 8
B, C, H, W = 8, 512, 32, 32
N = H * W                      # 1024 spatial positions per image
CMID = 256                     # conv1 output channels
HP = H + 2                     # padded spatial dim (34)
NPAD = HP * HP                 # 1156 elems per channel chunk in y1c
BN_EPS = 1e-5
LRELU_SLOPE = 0.01
M_TOTAL = float(B * N)         # BN statistic count (full batch)

KC = C // 128                  # 4 channel chunks of x
KM = CMID // 128               # 2 channel chunks of mid features


def build_kernel():
    nc = bacc.Bacc("TRN2", target_bir_lowering=False, debug=False,
                   num_devices=NCORES)

    # ---- DRAM I/O -------------------------------------------------------
    x1d = nc.dram_tensor("x1s", [128, 4096], F32, kind="ExternalInput")
    x2d = nc.dram_tensor("x2s", [128, 4096], F32, kind="ExternalInput")
    xd = nc.dram_tensor("xs", [128, 4096], F32, kind="ExternalInput")
    w1d = nc.dram_tensor("w1t", [128, 1024], F32, kind="ExternalInput")
    w2d = nc.dram_tensor("w2t", [128, 9216], F32, kind="ExternalInput")
    # vecs rows: 0=b2, 1=gamma, 2=bn_bias, 3=beta(col0), 4=b1(cols 0:256)
    vecd = nc.dram_tensor("vecs", [8, 512], F32, kind="ExternalInput")
    outd = nc.dram_tensor("out", [128, 4096], F32, kind="ExternalOutput")

    cc_in = nc.dram_tensor("cc_in", [1, 2048], F32, kind="Internal")
    cc_out = nc.dram_tensor("cc_out", [1, 2048], F32, kind="Internal",
                            addr_space="Shared")
    cw_in = nc.dram_tensor("cw_in", [1, 8], F32, kind="Internal")
    cw_out = nc.dram_tensor("cw_out", [1, 8], F32, kind="Internal",
                            addr_space="Shared")

    with TileContext(nc, num_cores=NCORES) as tc:
        with (
            tc.tile_pool(name="const", bufs=1) as const,
            tc.tile_pool(name="big", bufs=1) as big,
            tc.tile_pool(name="work", bufs=2) as work,
            tc.tile_pool(name="vec", bufs=1) as vec,
            tc.tile_pool(name="ps", bufs=6, space="PSUM") as ps,
            tc.tile_pool(name="psS", bufs=2, space="PSUM") as psS,
        ):
            # ---- input DMAs first: conv1's operands lead ---------------
            w1t = big.tile([128, 1024], F32)
            nc.sync.dma_start(out=_r(w1t[:]), in_=_r(w1d[:]))
            x1s = work.tile([128, 4096], F32, tag="xin")
            nc.sync.dma_start(out=_r(x1s[:, 0:1024]), in_=_r(x1d[:, 0:1024]))
            nc.sync.dma_start(out=_r(x1s[:, 1024:2560]),
                              in_=_r(x1d[:, 1024:2560]))
            nc.sync.dma_start(out=_r(x1s[:, 2560:4096]),
                              in_=_r(x1d[:, 2560:4096]))

            # ---- constants / small tiles -------------------------------
            identity = const.tile([128, 128], F32)
            make_identity(nc, identity)
            ones_col = const.tile([128, 1], F32)   # lhsT for partition sums
            nc.vector.memset(ones_col[:], 1.0)
            nc.scalar.copy(_r(ones_col[:]), ones_col[:])
            ones_row = const.tile([1, 128], F32)   # K=1 lhsT (f32 mms)
            nc.vector.memset(ones_row[:], 1.0)
            ones_row_r = const.tile([1, 128], F32)  # K=1 lhsT (f32r mms)
            nc.vector.memset(ones_row_r[:], 1.0)
            nc.scalar.copy(_r(ones_row_r[:]), ones_row_r[:])
            one_one = const.tile([1, 1], F32)      # rhs for transpose-in mms
            nc.vector.memset(one_one[:], 1.0)
            eps_pp = const.tile([128, 1], F32)     # BN eps as bias APs
            nc.vector.memset(eps_pp[:], BN_EPS)
            eps_1 = const.tile([1, 1], F32)
            nc.vector.memset(eps_1[:], BN_EPS)
            zrow = const.tile([128, 32], F32)      # zero source for borders
            nc.vector.memset(zrow[:], 0.0)
            zrowf = const.tile([1, 8], F32)
            nc.vector.memset(zrowf[:], 0.0)
            # preload every ACT table now (idle head) so no 1.3us table
            # switch lands mid-stream on the critical path later
            tld = const.tile([1, 8], F32)
            nc.scalar.activation(tld[:], zrowf[:], AF.Exp)
            nc.scalar.activation(tld[:], zrowf[:], AF.Sqrt)
            nc.scalar.activation(tld[:], zrowf[:], AF.Square)
            nc.scalar.activation(tld[:], zrowf[:], AF.Lrelu,
                                 alpha=LRELU_SLOPE)

            # conv-critical inputs next: b1 (conv1 writes), b2 (conv2
            # bias), then the big w2t whose first chunk gates conv2 s=0
            b1pp = const.tile([128, KM], F32)      # b1 per-partition chunks
            for m in range(KM):
                nc.sync.dma_start(out=b1pp[:, m:m + 1],
                                  in_=vecd[4:5, 128 * m:128 * (m + 1)])
            b2row = const.tile([1, 512], F32)      # matmul rhs (f32r)
            nc.sync.dma_start(out=_r(b2row[:]), in_=_r(vecd[0:1, :]))
            w2t = big.tile([128, 9216], F32)
            for j in range(3):
                nc.sync.dma_start(out=_r(w2t[:, 3072 * j:3072 * (j + 1)]),
                                  in_=_r(w2d[:, 3072 * j:3072 * (j + 1)]))

            # b2 broadcast [128,512]: preloaded into each conv2 PSUM acc
            # by the DVE instead of a 512-row rank-1 matmul per s-chunk
            b2bps = ps.tile([128, 512], F32, tag="ps")
            nc.tensor.matmul(b2bps[:], _r(ones_row_r[:]), _r(b2row[:]),
                             start=True, stop=True)
            b2bcast = const.tile([128, 512], F32)
            nc.vector.tensor_copy(b2bcast[:], b2bps[:])

            # tiny warmup AllReduce: pays the collective setup cost while
            # the input DMAs stream, so the real stats ARs hit a warm path
            warm = const.tile([1, 8], F32)
            nc.vector.memset(warm[:], 1.0)
            nc.sync.dma_start(out=cw_in[:], in_=warm[:])
            nc.gpsimd.collective_compute(
                "AllReduce", ALU.add,
                replica_groups=[list(range(NCORES))],
                ins=[cw_in[:]], outs=[cw_out[:]])

            x2s = work.tile([128, 4096], F32, tag="xin")
            for j in range(2):
                nc.sync.dma_start(out=_r(x2s[:, 2048 * j:2048 * (j + 1)]),
                                  in_=_r(x2d[:, 2048 * j:2048 * (j + 1)]))
            # affine-math vectors are only needed post-AR1 (~100us in)
            gb = const.tile([1, 1024], F32)        # gamma | bn_bias (f32)
            nc.sync.dma_start(out=gb[0:1, 0:512], in_=vecd[1:2, :])
            nc.sync.dma_start(out=gb[0:1, 512:1024], in_=vecd[2:3, :])
            betar = const.tile([1, 1], F32)
            nc.sync.dma_start(out=betar[:], in_=vecd[3:4, 0:1])
            # gamma / bn_bias per-partition [128, 4] (partition-scatter)
            gpp = const.tile([128, KC], F32)
            nc.sync.dma_start(
                out=gpp[:],
                in_=vecd[1:2, :].rearrange("o (j p) -> o p j", p=128))
            bnbpp = const.tile([128, KC], F32)
            nc.sync.dma_start(
                out=bnbpp[:],
                in_=vecd[2:3, :].rearrange("o (j p) -> o p j", p=128))

            # conv1 output, 3 horizontally-pre-shifted copies (kw = 0,1,2),
            # each vertically zero-padded to 34 rows of 32 contiguous cols:
            #   y1c[kw][k][c, r, w] = Y1[c_chunk k][r-1, w + kw - 1]
            # so conv2's stationary operand (s, kh, kw) is the contiguous
            # 128-elem slice at rows 4s+kh .. 4s+kh+3 of copy kw.
            NROW = HP * W                       # 1088 elems per copy/chunk
            y1c = big.tile([128, 3 * KM * NROW], F32)

            def y1base(kw, k):
                return (kw * KM + k) * NROW

            def ycv(kw, k):
                return y1c[:, y1base(kw, k):y1base(kw, k) + NROW].rearrange(
                    "p (r c) -> p r c", c=W)
            # zero only the borders (interior is fully overwritten by conv1)
            zr = zrow[:].rearrange("p (a c) -> p a c", a=1)      # [128,1,32]
            zc = zrow[:].rearrange("p (c a) -> p c a", a=1)      # [128,32,1]
            for kw in range(3):
                for k in range(KM):
                    v = ycv(kw, k)
                    nc.scalar.copy(_r(v[:, 0:1, :]), zr)         # top row
                    nc.scalar.copy(_r(v[:, HP - 1:HP, :]), zr)   # bottom
                    if kw == 0:
                        nc.scalar.copy(_r(v[:, 1:HP - 1, 0:1]), zc)
                    if kw == 2:
                        nc.scalar.copy(_r(v[:, 1:HP - 1, W - 1:W]), zc)

            f1t = big.tile([128, 4096], F32)
            f2t = big.tile([128, 4096], F32)
            stats = const.tile([1, 2048], F32)     # local r1|s1|r2|s2
            r2lrow = const.tile([1, 512], F32)     # local r2, f32r for PE
            ar = const.tile([1, 2048], F32)        # all-reduced stats

            # ---- one conv block: x -> conv1 -> pad -> conv2 -> lrelu ---
            def conv_block(xin, ft, si):
                # conv1: Y1[cmid, n] = W1 @ x + b1, written into shifted
                # copies of y1c (n2-outer: conv2's early s-chunks only need
                # the n2=0 rows, so they start after 6 writes instead of 12)
                for n2 in range(2):
                    for m in range(KM):
                        acc = ps.tile([128, 512], F32, tag="ps")
                        for k in range(KC):
                            nc.tensor.matmul(
                                acc[:],
                                _r(w1t[:, 256 * k + 128 * m:256 * k + 128 * (m + 1)]),
                                _r(xin[:, 1024 * k + 512 * n2:1024 * k + 512 * (n2 + 1)]),
                                start=(k == 0), stop=(k == KC - 1))
                        accv = acc[:].rearrange("p (r c) -> p r c", c=W)
                        row0 = (1 + 16 * n2) * W
                        # center copy (kw=1): straight contiguous store
                        nc.scalar.activation(
                            _r(y1c[:, y1base(1, m) + row0:y1base(1, m) + row0 + 512]),
                            acc[:], AF.Identity, bias=b1pp[:, m:m + 1])
                        # kw=0: shift right one col (src cols 0..30 -> 1..31)
                        d0 = ycv(0, m)
                        nc.scalar.activation(
                            _r(d0[:, 1 + 16 * n2:17 + 16 * n2, 1:32]),
                            accv[:, :, 0:31], AF.Identity,
                            bias=b1pp[:, m:m + 1])
                        # kw=2: shift left one col (src cols 1..31 -> 0..30)
                        d2 = ycv(2, m)
                        nc.scalar.activation(
                            _r(d2[:, 1 + 16 * n2:17 + 16 * n2, 0:31]),
                            accv[:, :, 1:32], AF.Identity,
                            bias=b1pp[:, m:m + 1])

                # conv2 (3x3) -> transposed output F^T[n, c], s-outer so the
                # activations + stats interleave with the conv matmuls
                racc = psS.tile([1, 512], F32, tag="stat")
                qacc = psS.tile([1, 512], F32, tag="stat")
                for s in range(8):
                    acc = ps.tile([128, 512], F32, tag="ps")
                    # bias: + b2[c] on every row, written by the DVE so the
                    # PE skips a 512-row rank-1 matmul per s-chunk
                    nc.vector.tensor_copy(acc[:], b2bcast[:])
                    for kh in range(3):
                        for kw in range(3):
                            t = kh * 3 + kw
                            for k in range(KM):
                                off = y1base(kw, k) + (4 * s + kh) * W
                                rhs = w2t[:, (2 * t + k) * 512:(2 * t + k + 1) * 512]
                                last = (kh == 2 and kw == 2 and k == KM - 1)
                                nc.tensor.matmul(acc[:],
                                                 _r(y1c[:, off:off + 128]),
                                                 _r(rhs),
                                                 start=False, stop=last,
                                                 skip_group_check=True)
                    nc.scalar.activation(_r(ft[:, 512 * s:512 * (s + 1)]),
                                         acc[:], AF.Lrelu,
                                         alpha=LRELU_SLOPE)
                    sq = work.tile([128, 512], F32, tag="sq")
                    nc.scalar.activation(_r(sq[:]),
                                         ft[:, 512 * s:512 * (s + 1)],
                                         AF.Square)
                    nc.tensor.matmul(racc[:], _r(ones_col[:]),
                                     _r(ft[:, 512 * s:512 * (s + 1)]),
                                     start=(s == 0), stop=(s == 7))
                    nc.tensor.matmul(qacc[:], _r(ones_col[:]), _r(sq[:]),
                                     start=(s == 0), stop=(s == 7))

                # local stats -> sbuf + AllReduce for this block
                nc.scalar.copy(stats[0:1, 1024 * si:1024 * si + 512], racc[:])
                nc.scalar.copy(stats[0:1, 1024 * si + 512:1024 * (si + 1)],
                               qacc[:])
                if si == 1:
                    nc.scalar.copy(r2lrow[:], racc[:])
                nc.sync.dma_start(out=cc_in[0:1, 1024 * si:1024 * (si + 1)],
                                  in_=stats[0:1, 1024 * si:1024 * (si + 1)])
                nc.gpsimd.collective_compute(
                    "AllReduce", ALU.add,
                    replica_groups=[list(range(NCORES))],
                    ins=[cc_in[0:1, 1024 * si:1024 * (si + 1)]],
                    outs=[cc_out[0:1, 1024 * si:1024 * (si + 1)]])
                if si == 1:
                    nc.sync.dma_start(out=ar[0:1, 1024:2048],
                                      in_=cc_out[0:1, 1024:2048])

            def tail(xs, r1pp, s1pp):
                # ---- raw Gram matmuls (no AR dependency) ---------------
                ssb = big.tile([128, 2048], F32)
                for m in range(KC):
                    sacc = ps.tile([128, 512], F32, tag="ps")
                    for k in range(8):
                        nc.tensor.matmul(
                            sacc[:],
                            _r(f1t[:, 512 * k + 128 * m:512 * k + 128 * (m + 1)]),
                            _r(f2t[:, 512 * k:512 * (k + 1)]),
                            start=(k == 0), stop=(k == 7))
                    nc.vector.tensor_copy(ssb[:, 512 * m:512 * (m + 1)],
                                          sacc[:])

                # ---- block-1 affine params, per-partition layout (AR1
                # landed long ago; runs on DVE concurrently with the Gram;
                # the scatter DMAs were issued between the conv blocks so
                # they are not queued behind the AR2-blocked ar dma) -------
                a1pp = vec.tile([128, KC], F32)
                nc.vector.tensor_scalar_mul(r1pp[:], r1pp[:], 1.0 / M_TOTAL)
                nc.vector.tensor_mul(a1pp[:], r1pp[:], r1pp[:])    # mean^2
                nc.vector.tensor_scalar(s1pp[:], s1pp[:], 1.0 / M_TOTAL,
                                        BN_EPS, op0=ALU.mult, op1=ALU.add)
                nc.vector.tensor_sub(s1pp[:], s1pp[:], a1pp[:])    # var+eps
                nc.scalar.activation(s1pp[:], s1pp[:], AF.Sqrt)
                nc.vector.reciprocal(s1pp[:], s1pp[:])
                nc.vector.tensor_mul(a1pp[:], gpp[:], s1pp[:])
                # b1bn per-partition = bn_bias - mean1*a1 (r1pp holds mean1)
                b1bnpp = vec.tile([128, KC], F32)
                nc.vector.tensor_mul(b1bnpp[:], r1pp[:], a1pp[:])
                nc.vector.tensor_sub(b1bnpp[:], bnbpp[:], b1bnpp[:])

                # local r1, r2 into per-partition layout (pre-AR2, idle PE)
                r2lps = ps.tile([128, 2 * KC], F32, tag="ps")
                for j in range(KC):
                    nc.tensor.matmul(r2lps[:, j:j + 1],
                                     stats[0:1, 128 * j:128 * (j + 1)],
                                     one_one[:], start=True, stop=True)
                    nc.tensor.matmul(r2lps[:, KC + j:KC + j + 1],
                                     r2lrow[0:1, 128 * j:128 * (j + 1)],
                                     one_one[:], start=True, stop=True)
                rlpp = vec.tile([128, 2 * KC], F32)
                nc.vector.tensor_copy(rlpp[:], r2lps[:])
                r2lpp = rlpp[:, KC:2 * KC]
                # u per-partition = a1 * r1_local
                u_pp = vec.tile([128, KC], F32)
                nc.vector.tensor_mul(u_pp[:], a1pp[:], rlpp[:, 0:KC])

                tld2 = vec.tile([1, 8], F32)
                # beta broadcast [128,1]
                bps = ps.tile([128, 1], F32, tag="ps")
                nc.tensor.matmul(bps[:], ones_row[:], betar[:],
                                 start=True, stop=True)
                betapp = vec.tile([128, 1], F32)
                nc.vector.tensor_copy(betapp[:], bps[:])

                # HAM keep-warm fillers bridging the AR2 latency window
                # (~0.26us each; the recent-measured collective latency is
                # 20-30us and the Gram covers only ~8.5us of it)
                for _ in range(48):
                    wf = psS.tile([128, 512], F32, tag="stat")
                    nc.tensor.matmul(wf[:], _r(ones_row_r[:]), _r(b2row[:]),
                                     start=True, stop=True)

                # scheduler fence: nothing below may be reordered above this
                # point, so the AR2-dependent matmuls cannot hoist ahead and
                # stall the in-order PE queue
                tc.no_sync_barrier()

                # ---- block-2 affine params, per-partition pipeline ------
                # transpose-in: ar2 (r2|s2 global) -> [128, 8] psum
                ppb = ps.tile([128, 8], F32, tag="ps")
                for j in range(8):
                    nc.tensor.matmul(ppb[:, j:j + 1],
                                     ar[0:1, 1024 + 128 * j:1024 + 128 * (j + 1)],
                                     one_one[:], start=True, stop=True)
                pps = vec.tile([128, 8], F32)
                nc.vector.tensor_copy(pps[:], ppb[:])
                r2pp = pps[:, 0:4]
                s2pp = pps[:, 4:8]
                # ppout cols: 0:4 a2 | 4:8 b2bn | 8:12 w
                ppout = vec.tile([128, 12], F32)
                ppt = vec.tile([128, 8], F32)
                mean2 = ppt[:, 0:4]
                var2 = ppt[:, 4:8]
                nc.vector.tensor_scalar_mul(mean2, r2pp, 1.0 / M_TOTAL)
                nc.vector.tensor_mul(var2, mean2, mean2)
                nc.vector.scalar_tensor_tensor(
                    var2, s2pp, 1.0 / M_TOTAL, var2,
                    op0=ALU.mult, op1=ALU.subtract)
                sd2 = ppout[:, 8:12]
                nc.scalar.activation(sd2, var2, AF.Sqrt, bias=eps_pp[:])
                nc.scalar.activation(tld2[:], zrowf[:], AF.Exp)
                nc.vector.reciprocal(var2, sd2)                    # rstd
                a2pp = ppout[:, 0:4]
                b2pp = ppout[:, 4:8]
                wpp = ppout[:, 8:12]
                nc.vector.tensor_mul(a2pp, var2, gpp[:])           # a2
                nc.vector.tensor_mul(var2, mean2, a2pp)            # m*a2
                nc.vector.tensor_sub(b2pp, bnbpp[:], var2)         # b2bn
                nc.vector.tensor_mul(var2, a2pp, r2lpp[:])         # a2*r2loc
                nc.vector.scalar_tensor_tensor(
                    wpp, b2pp, float(N), var2,
                    op0=ALU.mult, op1=ALU.add)                     # w
                # a2, b2bn, w -> free-layout [1,512] rows (4 matmuls each):
                # out[0, 128j+q] = sum_p ppout[p, col] * I[p, q]
                rows3 = vec.tile([1, 1536], F32)   # (a2) | b2bn | w
                a2row = vec.tile([1, 512], F32)    # a2, f32r-written
                for v in range(3):
                    vps = ps.tile([1, 512], F32, tag="ps")
                    for j in range(KC):
                        nc.tensor.matmul(vps[0:1, 128 * j:128 * (j + 1)],
                                         ppout[:, 4 * v + j:4 * v + j + 1],
                                         identity[:],
                                         start=True, stop=True)
                    if v == 0:
                        nc.scalar.copy(_r(a2row[:]), vps[:])
                    else:
                        nc.scalar.copy(rows3[0:1, 512 * v:512 * (v + 1)],
                                       vps[:])

                # broadcasts of a2 / b2bn / w to [128,512] (exact f32 mms;
                # copied to SBUF so the PSUM banks free up for the m-loop)
                bc3 = vec.tile([128, 1536], F32)   # a2b | b2bc | wbc
                for v in range(3):
                    bcp = ps.tile([128, 512], F32, tag="ps")
                    if v == 0:
                        nc.tensor.matmul(bcp[:], _r(ones_row_r[:]),
                                         _r(a2row[:]),
                                         start=True, stop=True)
                    else:
                        nc.tensor.matmul(bcp[:], ones_row[:],
                                         rows3[0:1, 512 * v:512 * (v + 1)],
                                         start=True, stop=True)
                    nc.vector.tensor_copy(bc3[:, 512 * v:512 * (v + 1)],
                                          bcp[:])
                a2b = bc3[:, 0:512]
                b2bc = bc3[:, 512:1024]
                wbc = bc3[:, 1024:1536]
                # keep the PE warm while the m=0 DVE chain fills
                for _ in range(12):
                    wf = psS.tile([128, 512], F32, tag="stat")
                    nc.tensor.matmul(wf[:], _r(ones_row_r[:]), _r(b2row[:]),
                                     start=True, stop=True)

                # ---- scores + softmax-exp + transpose + apply ----------
                et = y1c                 # E^T overlaid on dead y1c
                scvec = vec.tile([128, KC], F32)
                for m in range(KC):
                    # rank-1 score corrections (exact f32):
                    #   rk = u[c]*b2bn[d] + b1bn[c]*w[d]
                    rk = work.tile([128, 512], F32, tag="rk")
                    nc.vector.tensor_scalar(rk[:], b2bc, u_pp[:, m:m + 1],
                                            None, op0=ALU.mult)
                    nc.vector.scalar_tensor_tensor(
                        rk[:], wbc, b1bnpp[:, m:m + 1], rk[:],
                        op0=ALU.mult, op1=ALU.add)
                    # scores = (S * a2[d]) * a1[c] + rk  (the plain mul runs
                    # on GpSimd to keep the DVE free for the softmax chain)
                    tmul = work.tile([128, 512], F32, tag="tmul")
                    nc.gpsimd.tensor_mul(tmul[:], ssb[:, 512 * m:512 * (m + 1)],
                                         a2b)
                    sc = work.tile([128, 512], F32, tag="scores")
                    nc.vector.scalar_tensor_tensor(
                        sc[:], tmul[:], a1pp[:, m:m + 1], rk[:],
                        op0=ALU.mult, op1=ALU.add)
                    # E = exp(scores - rowmax), sumexp accumulated for free
                    nmx = vec.tile([128, 1], F32, tag="nmx")
                    nc.vector.tensor_reduce(nmx[:], sc[:],
                                            axis=mybir.AxisListType.X,
                                            op=ALU.max, negate=True)
                    esum = vec.tile([128, 1], F32, tag="esum")
                    ee = work.tile([128, 512], F32, tag="ee")
                    nc.scalar.activation(ee[:], sc[:], AF.Exp, bias=nmx[:],
                                         accum_out=esum[:])
                    nc.vector.reciprocal(esum[:], esum[:])
                    nc.vector.tensor_mul(scvec[:, m:m + 1], esum[:], betapp[:])
                    # transpose E chunk into et
                    for j in range(KC):
                        tp = ps.tile([128, 128], F32, tag="ps")
                        nc.tensor.transpose(tp[:], ee[:, 128 * j:128 * (j + 1)],
                                            identity[:])
                        nc.scalar.copy(
                            _r(et[:, 512 * j + 128 * m:512 * j + 128 * (m + 1)]),
                            tp[:])
                    # out[c, n] = (beta/sumexp)[c] * sum_d E^T[d,c] x[d,n]
                    for n2 in range(2):
                        oacc = ps.tile([128, 512], F32, tag="ps")
                        for k in range(KC):
                            nc.tensor.matmul(
                                oacc[:],
                                _r(et[:, 512 * k + 128 * m:512 * k + 128 * (m + 1)]),
                                _r(xs[:, 1024 * k + 512 * n2:1024 * k + 512 * (n2 + 1)]),
                                start=(k == 0), stop=(k == KC - 1))
                        ot = work.tile([128, 512], F32, tag="ot")
                        nc.scalar.mul(ot[:], oacc[:], scvec[:, m:m + 1])
                        nc.sync.dma_start(
                            out=outd[:, 1024 * m + 512 * n2:1024 * m + 512 * (n2 + 1)],
                            in_=ot[:])
                    if m < KC - 1:
                        for _ in range(8):
                            wf = psS.tile([128, 512], F32, tag="stat")
                            nc.tensor.matmul(wf[:], _r(ones_row_r[:]),
                                             _r(b2row[:]),
                                             start=True, stop=True)

            conv_block(x1s, f1t, 0)
            # x reuses x1's slot (x1 is dead after its conv1)
            xs = work.tile([128, 4096], F32, tag="xin")
            for j in range(2):
                nc.sync.dma_start(out=_r(xs[:, 2048 * j:2048 * (j + 1)]),
                                  in_=_r(xd[:, 2048 * j:2048 * (j + 1)]))
            # block-1 global-stats scatter loads, issued HERE so they sit
            # ahead of AR2's blocking ar-dma in the in-order sync queue
            # (they only wait on AR1, which completes during conv_block(x2))
            r1pp = vec.tile([128, KC], F32)
            s1pp = vec.tile([128, KC], F32)
            nc.sync.dma_start(
                out=r1pp[:],
                in_=cc_out[0:1, 0:512].rearrange("o (j p) -> o p j", p=128))
            nc.sync.dma_start(
                out=s1pp[:],
                in_=cc_out[0:1, 512:1024].rearrange("o (j p) -> o p j", p=128))
            conv_block(x2s, f2t, 1)
            tail(xs, r1pp, s1pp)

    nc.compile()
    return nc


_NC_CACHE = []


def _get_nc():
    if not _NC_CACHE:
        _NC_CACHE.append(build_kernel())
    return _NC_CACHE[0]


def _prep_shared(w1, b1, w2, b2, gamma, bn_bias, beta):
    w1m = w1.reshape(CMID, C).astype(np.float32)
    w1t = np.ascontiguousarray(
        w1m.T.reshape(KC, 128, CMID).transpose(1, 0, 2).reshape(128, KC * CMID))
    w2t = np.empty((128, 9216), dtype=np.float32)
    for kh in range(3):
        for kw in range(3):
            t = kh * 3 + kw
            wt = w2[:, :, kh, kw].T  # [256 in, 512 out]
            for k in range(KM):
                w2t[:, (2 * t + k) * 512:(2 * t + k + 1) * 512] = \
                    wt[128 * k:128 * (k + 1), :]
    vecs = np.zeros((8, 512), dtype=np.float32)
    vecs[0] = b2
    vecs[1] = gamma
    vecs[2] = bn_bias
    vecs[3, 0] = np.asarray(beta).reshape(-1)[0]
    vecs[4, :CMID] = b1
    return w1t, w2t, vecs


def _chunk_img(img):
    # [512, 1024] -> [128, 4096] with channel chunk k at cols 1024k
    return np.ascontiguousarray(
        img.reshape(KC, 128, N).transpose(1, 0, 2).reshape(128, KC * N))


def kernel(x, x1, x2, w1, b1, w2, b2, gamma, bn_bias, beta, **run_kw):
    nc = _get_nc()
    w1t, w2t, vecs = _prep_shared(w1, b1, w2, b2, gamma, bn_bias, beta)
    in_maps = []
    for i in range(NCORES):
        in_maps.append({
            "x1s": _chunk_img(np.asarray(x1[i], np.float32).reshape(C, N)),
            "x2s": _chunk_img(np.asarray(x2[i], np.float32).reshape(C, N)),
            "xs": _chunk_img(np.asarray(x[i], np.float32).reshape(C, N)),
            "w1t": w1t, "w2t": w2t, "vecs": vecs,
        })
    res = run_bass_kernel_spmd(nc, in_maps, list(range(NCORES)), **run_kw)
    out = np.empty((B, C, H, W), dtype=np.float32)
    for i in range(NCORES):
        o = res.results[i]["out"]  # [128, 4096]
        out[i] = o.reshape(128, KC, N).transpose(1, 0, 2).reshape(C, H, W)
    if run_kw:
        kernel.last_results = res
    return out
